# revision 63
# baseline (speedup 1.0000x reference)
"""Trainium2 Bass kernel for sparse (causal, tanh-clamped) attention.

Problem: B=2, L=2048, D=1024, H=16 heads x 64 dim; S = QK^T/8;
S = 30*tanh(S); causal + attention_mask; softmax; out = attn @ V.

Sharding: 2 heads per core across 8 cores (tensor-parallel on heads).

v2 design (ACT-engine-bound):
 - The scalar/ACT engine must run tanh+exp over every causal S element
   (~69.6K 128-lane columns per core); everything else is organized to
   hide under that ~138us floor.
 - Dual-head Q/K tiles [128, L] (head h on partitions 64h..64h+64); no
   augmentation row.  S^T = K^T @ Q per (head, k-tile) with contraction
   64 on partitions.
 - The causal column stream of each (b,h) unit (sum of widths
   2048-128*ki = 17408 = 17*1024) is packed into exactly 17 PSUM strips
   of [128, 1024] (2 banks each, double buffered): one tanh (in-place,
   PSUM) + one exp (PSUM -> SBUF bf16) per strip minimizes ACT
   per-instruction overhead.
 - AV runs TRANSPOSED: out[q, e] = pp_block[k, q].T @ vaug[k, 65] with
   the P block as the (bf16) stationary and V^T-augmented tiles as the
   65-wide bf16 moving operand (bf16 = 1 cyc/row at any width).  The
   result lands directly in [query, feature] layout: no transpose-back
   epilogue; normalization is one reciprocal + one tensor_scalar per
   128-query tile.
 - attention_mask is applied by scaling rows of the V^T tiles (and the
   appended denominator column holds the mask value itself): exact, and
   costs nothing on ACT/PE.
 - PSUM (8 banks): 2x2 strip + 1 proj + 3 po accumulators.  po packs
   the 16 [128,65] per-q-tile accumulators 7-per-bank so no tile
   crosses a bank (PSUM start=True zeroes whole 2KB banks; first
   toucher per bank sets start, later tiles ride the pending-zero).
 - First unit runs its k-tiles in REVERSE so attention starts after
   only the last 512-token group is projected; remaining projections
   and batch-1 work are pumped between strips.
"""

import sys

if "/opt/trn_rl_repo" not in sys.path:
    sys.path.insert(0, "/opt/trn_rl_repo")

import numpy as np

B = 2
L = 2048
D = 1024
N_CORES = 8
T = B * L            # 4096 tokens
E = 128              # per-core output features (2 heads)
TAU = 30.0
NK = L // 128        # 16 k tiles per sequence
NG = L // 512        # 4 512-token groups per batch
UCOLS = 17408        # per-unit causal stream columns = 17*1024
NSTRIP = UCOLS // 1024

_CACHE = {}


def _unit_schedule(reverse):
    """Static schedule of one (b, h) unit's causal column stream.

    strips[s] has:
      pieces: [(c0, c1, ki, qstart)]  S-matmul pieces (512-grid cut)
      diag:   [(c, ki)]               tril blocks at stream col c
      av:     [(c, ki, qt)]           AV blocks
      done:   [qt]                    q-tiles whose accumulation completes
    """
    kis = list(range(NK - 1, -1, -1)) if reverse else list(range(NK))
    strips = [dict(pieces=[], diag=[], av=[], done=[]) for _ in range(NSTRIP)]
    c = 0
    for ki in kis:
        q0, w = 128 * ki, L - 128 * ki
        a = c
        while a < c + w:
            b_ = min(c + w, (a // 512 + 1) * 512)
            strips[a // 1024]["pieces"].append((a, b_, ki, q0 + (a - c)))
            a = b_
        strips[c // 1024]["diag"].append((c, ki))
        for qt in range(ki, NK):
            blk = c + 128 * (qt - ki)
            strips[blk // 1024]["av"].append((blk, ki, qt))
        c += w
    for s in strips:
        for blk, ki, qt in s["av"]:
            if (not reverse and ki == qt) or (reverse and ki == 0):
                s["done"].append(qt)
    return strips


def _build_program():
    import concourse.bacc as bacc
    import concourse.tile as tile
    from concourse import mybir

    F32 = mybir.dt.float32
    F32R = mybir.dt.float32r
    BF16 = mybir.dt.bfloat16
    AF = mybir.ActivationFunctionType

    nc = bacc.Bacc("TRN2", target_bir_lowering=False, debug=False,
                   num_devices=N_CORES)

    xT_d = nc.dram_tensor("xT", [D, T], BF16, kind="ExternalInput")
    wq_d = nc.dram_tensor("wq", [128, (D // 128) * E], BF16, kind="ExternalInput")
    wk_d = nc.dram_tensor("wk", [128, (D // 128) * E], BF16, kind="ExternalInput")
    wv_d = nc.dram_tensor("wv", [128, (D // 128) * E], BF16, kind="ExternalInput")
    tril_d = nc.dram_tensor("tril", [128, 128], BF16, kind="ExternalInput")
    maskf_d = nc.dram_tensor("maskf", [128, B * NK], F32, kind="ExternalInput")
    out_d = nc.dram_tensor("out", [B, L, E], F32, kind="ExternalOutput")

    ND = D // 128        # 8 contraction chunks for projections

    with tile.TileContext(nc) as tc:
        with (
            tc.tile_pool(name="const", bufs=1) as constp,
            tc.tile_pool(name="weights", bufs=1) as wp,
            tc.tile_pool(name="qkv", bufs=1) as qkvp,
            tc.tile_pool(name="xin", bufs=8) as xp,
            tc.tile_pool(name="pp", bufs=2) as ppp,
            tc.tile_pool(name="epi", bufs=4) as epip,
            tc.tile_pool(name="ostage", bufs=1) as ostagep,
            tc.tile_pool(name="ps", bufs=1, space="PSUM") as psp,
        ):
            tril_t = constp.tile([128, 128], BF16, tag="tril")
            maskt = constp.tile([128, B * NK], F32, tag="maskt")
            n30_t = constp.tile([128, 1], F32, tag="n30")
            nc.gpsimd.memset(n30_t[:], -TAU)
            # Dummy activation at t~0: pulls the ACT table load (1.28us)
            # off the first strip's critical path.
            dummy_t = constp.tile([128, 1], F32, tag="dummy")
            with tc.high_priority():
                nc.scalar.activation(dummy_t[:], n30_t[:], AF.Tanh)

            # weight tiles: w[:, d*128:(d+1)*128] = W.T chunk d ([128, 128])
            # DMA order tuned for the startup ramp: wq first (Q matmuls of
            # the first group pipeline with the x loads), x group 3 next
            # (emitted inside prologue), then wk/wv/consts.
            w_tiles = []
            for name, d_in in (("wq", wq_d), ("wk", wk_d), ("wv", wv_d)):
                wt = wp.tile([128, ND * E], BF16, tag=name, name=name)
                w_tiles.append(wt)

            def load_w(p, d_in):
                nc.sync.dma_start(w_tiles[p][:], d_in.ap()[:])

            load_w(0, wq_d)

            def load_consts():
                load_w(1, wk_d)
                load_w(2, wv_d)
                nc.sync.dma_start(tril_t[:], tril_d.ap()[:])
                nc.sync.dma_start(maskt[:], maskf_d.ap()[:])

            # Per-batch dual-head QKV storage ([128, L]; head h at
            # partitions 64h..64h+64).
            QT = [qkvp.tile([128, L], F32R, tag=f"qt{b}", name=f"qt{b}")
                  for b in range(B)]
            KT = [qkvp.tile([128, L], F32R, tag=f"kt{b}", name=f"kt{b}")
                  for b in range(B)]
            # V^T-augmented tiles, bf16: per (ki, h) a [128, 65] block at
            # col (2ki+h)*65; col 64 = mask value (denominator column).
            VA = [qkvp.tile([128, 2 * NK * 65], BF16, tag=f"va{b}",
                            name=f"va{b}") for b in range(B)]

            def va_mask_cols(b):
                va_k = VA[b][:].rearrange("p (k c) -> p k c", k=NK)
                for h in range(2):
                    nc.vector.tensor_copy(
                        va_k[:, :, 65 * h + 64:65 * h + 65],
                        maskt[:, b * NK:(b + 1) * NK].rearrange(
                            "p k -> p k ()"))

            def load_group(b, g, pieces=2):
                """One 512-token group's x tiles as a few fused DMAs — one
                HWDGE descriptor-gen per piece instead of 8 (625ns cadence
                each).  More pieces -> matmuls start earlier (ramp)."""
                g0 = b * L + g * 512
                xt = xp.tile([128, ND * 512], BF16, tag="xt", name="xt",
                             bufs=8)
                dper = ND // pieces
                w_ = dper * 512
                for h in range(pieces):
                    nc.sync.dma_start(
                        xt[:, h * w_:(h + 1) * w_].rearrange(
                            "p (d t) -> p d t", d=dper),
                        xT_d.ap()[dper * 128 * h:dper * 128 * (h + 1),
                                  g0:g0 + 512].rearrange(
                            "(d p) t -> p d t", p=128))
                return [xt[:, d * 512:(d + 1) * 512] for d in range(ND)]

            def queue_vdir(b, g):
                """V^T computed directly: out[tok, feat] accumulates
                stationary x-chunks [d,128tok] against moving W_V chunks
                [d,128feat] (bf16 = 1 cyc at 128 wide).  No VT staging, no
                transposes; the mask-scaled copy into VA reads PSUM from
                GPSIMD."""
                shared_pv = {}

                def mk(half):
                    def f():
                        xts = gshared[(b, g)]["x"]
                        if half == 0:
                            shared_pv["t"] = psp.tile(
                                [128, 512], F32, tag="proj", name="pvt")
                        pvt = shared_pv["t"]
                        for tl in range(2 * half, 2 * half + 2):
                            for d in range(ND):
                                nc.tensor.matmul(
                                    pvt[:, tl * 128:(tl + 1) * 128],
                                    xts[d][:, tl * 128:(tl + 1) * 128],
                                    w_tiles[2][:, d * E:(d + 1) * E],
                                    start=(tl == 0 and d == 0),
                                    stop=(d == ND - 1))
                        for tl in range(2 * half, 2 * half + 2):
                            for h in range(2):
                                ki = 4 * g + tl
                                nc.vector.tensor_scalar_mul(
                                    VA[b][:, (2 * ki + h) * 65:
                                           (2 * ki + h) * 65 + 64],
                                    pvt[:, tl * 128 + 64 * h:
                                        tl * 128 + 64 * h + 64],
                                    maskt[:, b * NK + ki:b * NK + ki + 1])
                    return f

                queue(f"b{b}g{g}vda", mk(0))
                queue(f"b{b}g{g}va", mk(1))

            # ---------------- background work pump ----------------------
            from collections import deque
            pending = deque()
            emitted = set()

            def pump(n=1):
                for _ in range(n):
                    if pending:
                        label, unit = pending.popleft()
                        unit()
                        emitted.add(label)

            def ensure(label):
                while pending and label not in emitted:
                    pump()

            def queue(label, fn):
                pending.append((label, fn))

            def inline_load(b, g):
                gshared[(b, g)] = {"x": load_group(b, g)}

            def queue_phase(b, g, p):
                def mk_half(half):
                    def f():
                        shared = gshared[(b, g)]
                        if half == 0:
                            shared[p] = psp.tile([128, 512], F32,
                                                 tag="proj", name="pj")
                        pj = shared[p]
                        for d in range(4 * half, 4 * half + 4):
                            nc.tensor.matmul(
                                pj[:], w_tiles[p][:, d * E:(d + 1) * E],
                                shared["x"][d][:],
                                start=(d == 0), stop=(d == ND - 1))
                        if half == 1:
                            dst = (QT, KT)[p][b]
                            with tc.high_priority(offset=200):
                                nc.vector.tensor_copy(
                                    dst[:, g * 512:(g + 1) * 512], pj[:])
                    return f

                for half in range(2):
                    queue(f"b{b}g{g}p{p}{'ab'[half]}", mk_half(half))

            gshared = {}

            # ---------------- attention unit ----------------------------
            def attention_unit(b, h, reverse, deps, post_deps=None,
                               hook0=None, hook_pre=None, lag=True):
                strips = _unit_schedule(reverse)
                pp = ppp.tile([128, UCOLS], BF16, tag="pp", name="pp")
                po = psp.tile([128, 1536], F32, tag="po", name="po")
                po_touched = [False, False, False]

                def po_slice(qt):
                    bank, j = divmod(qt, 7)
                    c0 = bank * 512 + j * 65
                    return po[:, c0:c0 + 65], bank

                st_tiles = [None] * NSTRIP

                def emit_S(s):
                    # Rank S matmuls ahead of recently-queued pump work in
                    # the tile list-scheduler: tanh(s) waits only on these.
                    # No boost in the first unit's ramp: there the emission
                    # order (g3 proj, g2 proj, S) is already the schedule.
                    boost = 0 if (reverse and s < 3) else 200
                    ctx_p = tc.high_priority(offset=boost)
                    ctx_p.__enter__()
                    st = psp.tile([128, 1024], F32, tag="strip", name="st",
                                  bufs=2)
                    st_tiles[s] = st
                    touched = [False, False]
                    for (c0, c1, ki, qs) in strips[s]["pieces"]:
                        o0 = c0 % 1024
                        bank = o0 // 512
                        nc.tensor.matmul(
                            st[:, o0:o0 + (c1 - c0)],
                            KT[b][h * 64:h * 64 + 64,
                                  ki * 128:(ki + 1) * 128],
                            QT[b][h * 64:h * 64 + 64, qs:qs + (c1 - c0)],
                            start=not touched[bank], stop=True)
                        touched[bank] = True
                    ctx_p.__exit__(None, None, None)

                def emit_tanh(s):
                    st = st_tiles[s]
                    nc.scalar.activation(st[:], st[:], AF.Tanh, scale=0.125)

                def emit_exp(s):
                    st = st_tiles[s]
                    nc.scalar.activation(pp[:, s * 1024:(s + 1) * 1024],
                                         st[:], AF.Exp, bias=n30_t[:],
                                         scale=TAU)
                    st_tiles[s] = None

                def emit_post(s):
                    for (c, ki) in strips[s]["diag"]:
                        nc.vector.tensor_mul(pp[:, c:c + 128],
                                             pp[:, c:c + 128], tril_t[:])
                    for (c, ki, qt) in strips[s]["av"]:
                        dst, bank = po_slice(qt)
                        stop = (ki == qt) if not reverse else (ki == 0)
                        nc.tensor.matmul(
                            dst, pp[:, c:c + 128],
                            VA[b][:, (2 * ki + h) * 65:(2 * ki + h) * 65 + 65],
                            start=not po_touched[bank], stop=stop)
                        po_touched[bank] = True
                    for qt in strips[s]["done"]:
                        src, _ = po_slice(qt)
                        rec = epip.tile([128, 1], F32, tag="rec", name="rec")
                        nc.vector.reciprocal(rec[:], src[:, 64:65])
                        nc.vector.tensor_scalar_mul(
                            ostages[b][qt][:, h * 64:(h + 1) * 64],
                            src[:, 0:64], rec[:])
                        if h == 1:
                            eng = nc.gpsimd if (b == 1 and qt >= 14) else nc.sync
                            eng.dma_start(
                                out_d.ap()[b, qt * 128:(qt + 1) * 128, :],
                                ostages[b][qt][:])

                # Strips are processed in PAIRS with the ACT stream ordered
                # tanh(s), tanh(s+1), exp(s), exp(s+1): the RAW tanh->exp
                # semaphore wait (~177ns pipeline tail) then lands inside the
                # neighboring activation instead of idling the ACT engine.
                # The next pair's S matmuls are emitted first (each engine
                # runs its program in order; tanh waits only on S).
                # The post phase (tril + AV + drains) of each strip group is
                # emitted one group LATE: AV blocks never gate the ACT
                # stream, and the lag keeps the V-phase/va work (which AV
                # needs) off the S-matmul critical path during the ramp.
                groups = [[0]] + [[s, s + 1] for s in range(1, NSTRIP, 2)]
                post_deps = post_deps or {}

                def do_post(s):
                    for lbl in post_deps.get(s, []):
                        ensure(lbl)
                    emit_post(s)

                for i, grp in enumerate(groups):
                    nxt = groups[i + 1] if i + 1 < len(groups) else []
                    if i == 0:
                        if hook0 is not None:
                            hook0()
                        for s in grp:
                            for lbl in deps.get(s, []):
                                ensure(lbl)
                            emit_S(s)
                    if lag and i > 0:
                        for s in groups[i - 1]:
                            do_post(s)
                    for s in nxt:
                        for lbl in deps.get(s, []):
                            ensure(lbl)
                        emit_S(s)
                    pump()
                    for s in grp:
                        emit_tanh(s)
                    for s in grp:
                        emit_exp(s)
                    if not lag:
                        for s in grp:
                            do_post(s)
                    pump()
                if lag:
                    for s in groups[-1]:
                        do_post(s)

            ostages = [[ostagep.tile([128, 128], F32, tag=f"os{b}_{j}",
                                     name=f"os{b}_{j}")
                        for j in range(NK)] for b in range(B)]

            # ---------------- prologue: b0 groups 3, 2 -------------------
            # Q|K packed into one 2-bank strip tile; V in the proj bank;
            # then va transposes reuse the proj bank.
            # PE warmup: the cost model's p-state needs ~3us of continuous
            # matmul activity to reach full clock.  Burn idle PE time at
            # t=0 on throwaway matmuls so the first projections run warm.
            warm_sb = constp.tile([128, 256], F32R, tag="warm")
            nc.gpsimd.memset(warm_sb[:].bitcast(F32), 0.0)
            warm_ps = psp.tile([128, 512], F32, tag="proj", name="warm")
            for i in range(10):
                nc.tensor.matmul(warm_ps[:, 0:256], warm_sb[:, 0:128],
                                 warm_sb[:], start=True, stop=True)

            # Prologue: group 3 of batch 0 inline — Q|K packed into one
            # 2-bank strip tile, pipelined with the fused x DMAs; V and
            # the va transposes are deferred to the pump queue so the
            # first attention strips start as early as possible.
            def prologue_qk(b, g, act_drain=False):
                xts = load_group(b, g, pieces=4)
                gshared[(b, g)] = {"x": xts}
                st = psp.tile([128, 1024], F32, tag="strip", name="pqk",
                              bufs=2)
                for d in range(ND):
                    nc.tensor.matmul(st[:, 0:512],
                                     w_tiles[0][:, d * E:(d + 1) * E],
                                     xts[d][:], start=(d == 0),
                                     stop=(d == ND - 1))
                    nc.tensor.matmul(st[:, 512:1024],
                                     w_tiles[1][:, d * E:(d + 1) * E],
                                     xts[d][:], start=(d == 0),
                                     stop=(d == ND - 1))
                nc.vector.tensor_copy(QT[b][:, g * 512:(g + 1) * 512],
                                      st[:, 0:512])
                if act_drain:
                    # ACT is idle before the first strip: drain K there so
                    # it runs in parallel with Q's DVE drain.
                    nc.scalar.activation(KT[b][:, g * 512:(g + 1) * 512],
                                         st[:, 512:1024], AF.Identity)
                else:
                    nc.vector.tensor_copy(KT[b][:, g * 512:(g + 1) * 512],
                                          st[:, 512:1024])

            load_w(1, wk_d)
            prologue_qk(0, 3)
            nc.sync.dma_start(tril_t[:], tril_d.ap()[:])
            nc.sync.dma_start(maskt[:], maskf_d.ap()[:])
            va_mask_cols(0)
            va_mask_cols(1)

            # group 2 (packed QK) is emitted from inside unit 0, right
            # after strip 0's S matmuls, so S(0) stays ahead of it on the
            # in-order PE stream.
            def hook_g2():
                prologue_qk(0, 2)
                load_w(2, wv_d)
                inline_load(0, 1)
                inline_load(0, 0)
                for g in range(NG):
                    inline_load(1, g)

            # All remaining x loads go on the DMA queue back-to-back, up
            # front (the DMA device is the ramp bottleneck; consumers wait
            # on semaphores).  The pump queue holds only compute: g1's Q/K
            # before the g3/g2 V-phase work (S matmuls gate ACT; AV work
            # is post-lagged), b1 Q phases before b1 K/V/va.
            queue_phase(0, 1, 0)
            queue_phase(0, 1, 1)
            queue_phase(0, 0, 0)
            queue_phase(0, 0, 1)
            queue_vdir(0, 3)
            queue_vdir(0, 2)
            queue_vdir(0, 1)
            queue_vdir(0, 0)
            for g in range(NG):
                queue_phase(1, g, 0)
            for g in range(NG):
                queue_phase(1, g, 1)
                queue_vdir(1, g)

            rev_deps = {4: ["b0g1p1b"], 9: ["b0g0p1b"]}
            rev_post = {0: ["b0g3va"], 1: ["b0g2va"],
                        4: ["b0g1va"], 9: ["b0g0va"]}
            u2_deps = {0: ["b1g0p1b"], 7: ["b1g1p1b"],
                       12: ["b1g2p1b"], 15: ["b1g3p1b"]}
            u2_post = {0: ["b1g0va"], 7: ["b1g1va"],
                       12: ["b1g2va"], 15: ["b1g3va"]}

            attention_unit(0, 0, reverse=True, deps=rev_deps,
                           post_deps=rev_post, hook0=hook_g2)
            attention_unit(0, 1, reverse=False, deps={})
            attention_unit(1, 0, reverse=False, deps=u2_deps,
                           post_deps=u2_post)
            attention_unit(1, 1, reverse=False, deps={}, lag=False)
            while pending:
                pump()

    nc.compile()
    return nc


def _get_program():
    if "nc" not in _CACHE:
        _CACHE["nc"] = _build_program()
    return _CACHE["nc"]


def _wprep(W, sl):
    """Per-core weight slice in the SBUF tile layout [128, 8*128]
    (partition-major: wt[p, d*128+e] = W.T[128*d+p, e]) so the DMA is a
    straight wide-row copy (2KB runs, no small-element penalty)."""
    import ml_dtypes
    wt = np.asarray(W, np.float32)[sl, :].T.reshape(8, 128, 128)
    wt = wt.transpose(1, 0, 2).reshape(128, 1024)
    return np.ascontiguousarray(wt).astype(ml_dtypes.bfloat16)


def _prep_inputs(input, attention_mask, W_Q, W_K, W_V):
    import ml_dtypes

    x = np.asarray(input, dtype=np.float32).reshape(T, D)
    xT = np.ascontiguousarray(x.T).astype(ml_dtypes.bfloat16)   # [D, T]
    mask = np.asarray(attention_mask).astype(np.float32)    # [B, L]
    # maskf[p, b*16+ki] = mask[b, 128*ki + p]
    maskf = np.ascontiguousarray(
        mask.reshape(B, NK, 128).transpose(2, 0, 1).reshape(128, B * NK))
    tril = np.triu(np.ones((128, 128), dtype=np.float32))   # keep[k, q] = q >= k
    tril = tril.astype(ml_dtypes.bfloat16)
    common = {"xT": xT, "tril": tril, "maskf": maskf}
    in_maps = []
    for c in range(N_CORES):
        sl = slice(c * E, (c + 1) * E)
        in_maps.append({
            **common,
            "wq": _wprep(W_Q, sl),
            "wk": _wprep(W_K, sl),
            "wv": _wprep(W_V, sl),
        })
    return in_maps


def kernel(input, attention_mask, W_Q, W_K, W_V):
    from concourse.bass_utils import run_bass_kernel_spmd

    nc = _get_program()
    in_maps = _prep_inputs(input, attention_mask, W_Q, W_K, W_V)
    res = run_bass_kernel_spmd(nc, in_maps, list(range(N_CORES)))
    return np.concatenate([res.results[c]["out"] for c in range(N_CORES)],
                          axis=2)


# revision 65
# speedup vs baseline: 1.0028x; 1.0028x over previous
"""Trainium2 Bass kernel for sparse (causal, tanh-clamped) attention.

Problem: B=2, L=2048, D=1024, H=16 heads x 64 dim; S = QK^T/8;
S = 30*tanh(S); causal + attention_mask; softmax; out = attn @ V.

Sharding: 2 heads per core across 8 cores (tensor-parallel on heads).

v2 design (ACT-engine-bound):
 - The scalar/ACT engine must run tanh+exp over every causal S element
   (~69.6K 128-lane columns per core); everything else is organized to
   hide under that ~138us floor.
 - Dual-head Q/K tiles [128, L] (head h on partitions 64h..64h+64); no
   augmentation row.  S^T = K^T @ Q per (head, k-tile) with contraction
   64 on partitions.
 - The causal column stream of each (b,h) unit (sum of widths
   2048-128*ki = 17408 = 17*1024) is packed into exactly 17 PSUM strips
   of [128, 1024] (2 banks each, double buffered): one tanh (in-place,
   PSUM) + one exp (PSUM -> SBUF bf16) per strip minimizes ACT
   per-instruction overhead.
 - AV runs TRANSPOSED: out[q, e] = pp_block[k, q].T @ vaug[k, 65] with
   the P block as the (bf16) stationary and V^T-augmented tiles as the
   65-wide bf16 moving operand (bf16 = 1 cyc/row at any width).  The
   result lands directly in [query, feature] layout: no transpose-back
   epilogue; normalization is one reciprocal + one tensor_scalar per
   128-query tile.
 - attention_mask is applied by scaling rows of the V^T tiles (and the
   appended denominator column holds the mask value itself): exact, and
   costs nothing on ACT/PE.
 - PSUM (8 banks): 2x2 strip + 1 proj + 3 po accumulators.  po packs
   the 16 [128,65] per-q-tile accumulators 7-per-bank so no tile
   crosses a bank (PSUM start=True zeroes whole 2KB banks; first
   toucher per bank sets start, later tiles ride the pending-zero).
 - First unit runs its k-tiles in REVERSE so attention starts after
   only the last 512-token group is projected; remaining projections
   and batch-1 work are pumped between strips.
"""

import sys

if "/opt/trn_rl_repo" not in sys.path:
    sys.path.insert(0, "/opt/trn_rl_repo")

import numpy as np

B = 2
L = 2048
D = 1024
N_CORES = 8
T = B * L            # 4096 tokens
E = 128              # per-core output features (2 heads)
TAU = 30.0
NK = L // 128        # 16 k tiles per sequence
NG = L // 512        # 4 512-token groups per batch
UCOLS = 17408        # per-unit causal stream columns = 17*1024
NSTRIP = UCOLS // 1024

_CACHE = {}


def _unit_schedule(reverse):
    """Static schedule of one (b, h) unit's causal column stream.

    strips[s] has:
      pieces: [(c0, c1, ki, qstart)]  S-matmul pieces (512-grid cut)
      diag:   [(c, ki)]               tril blocks at stream col c
      av:     [(c, ki, qt)]           AV blocks
      done:   [qt]                    q-tiles whose accumulation completes
    """
    kis = list(range(NK - 1, -1, -1)) if reverse else list(range(NK))
    strips = [dict(pieces=[], diag=[], av=[], done=[]) for _ in range(NSTRIP)]
    c = 0
    for ki in kis:
        q0, w = 128 * ki, L - 128 * ki
        a = c
        while a < c + w:
            b_ = min(c + w, (a // 512 + 1) * 512)
            strips[a // 1024]["pieces"].append((a, b_, ki, q0 + (a - c)))
            a = b_
        strips[c // 1024]["diag"].append((c, ki))
        for qt in range(ki, NK):
            blk = c + 128 * (qt - ki)
            strips[blk // 1024]["av"].append((blk, ki, qt))
        c += w
    for s in strips:
        for blk, ki, qt in s["av"]:
            if (not reverse and ki == qt) or (reverse and ki == 0):
                s["done"].append(qt)
    return strips


def _build_program():
    import concourse.bacc as bacc
    import concourse.tile as tile
    from concourse import mybir

    F32 = mybir.dt.float32
    F32R = mybir.dt.float32r
    BF16 = mybir.dt.bfloat16
    AF = mybir.ActivationFunctionType

    nc = bacc.Bacc("TRN2", target_bir_lowering=False, debug=False,
                   num_devices=N_CORES)

    xT_d = nc.dram_tensor("xT", [D, T], BF16, kind="ExternalInput")
    wq_d = nc.dram_tensor("wq", [128, (D // 128) * E], BF16, kind="ExternalInput")
    wk_d = nc.dram_tensor("wk", [128, (D // 128) * E], BF16, kind="ExternalInput")
    wv_d = nc.dram_tensor("wv", [128, (D // 128) * E], BF16, kind="ExternalInput")
    tril_d = nc.dram_tensor("tril", [128, 128], BF16, kind="ExternalInput")
    maskf_d = nc.dram_tensor("maskf", [128, B * NK], F32, kind="ExternalInput")
    out_d = nc.dram_tensor("out", [B, L, E], F32, kind="ExternalOutput")

    ND = D // 128        # 8 contraction chunks for projections

    with tile.TileContext(nc) as tc:
        with (
            tc.tile_pool(name="const", bufs=1) as constp,
            tc.tile_pool(name="weights", bufs=1) as wp,
            tc.tile_pool(name="qkv", bufs=1) as qkvp,
            tc.tile_pool(name="xin", bufs=8) as xp,
            tc.tile_pool(name="pp", bufs=2) as ppp,
            tc.tile_pool(name="epi", bufs=4) as epip,
            tc.tile_pool(name="ostage", bufs=1) as ostagep,
            tc.tile_pool(name="ps", bufs=1, space="PSUM") as psp,
        ):
            tril_t = constp.tile([128, 128], BF16, tag="tril")
            maskt = constp.tile([128, B * NK], F32, tag="maskt")
            n30_t = constp.tile([128, 1], F32, tag="n30")
            nc.gpsimd.memset(n30_t[:], -TAU)
            # Dummy activation at t~0: pulls the ACT table load (1.28us)
            # off the first strip's critical path.
            dummy_t = constp.tile([128, 1], F32, tag="dummy")
            with tc.high_priority():
                nc.scalar.activation(dummy_t[:], n30_t[:], AF.Tanh)

            # weight tiles: w[:, d*128:(d+1)*128] = W.T chunk d ([128, 128])
            # DMA order tuned for the startup ramp: wq first (Q matmuls of
            # the first group pipeline with the x loads), x group 3 next
            # (emitted inside prologue), then wk/wv/consts.
            w_tiles = []
            for name, d_in in (("wq", wq_d), ("wk", wk_d), ("wv", wv_d)):
                wt = wp.tile([128, ND * E], BF16, tag=name, name=name)
                w_tiles.append(wt)

            def load_w(p, d_in):
                nc.sync.dma_start(w_tiles[p][:], d_in.ap()[:])

            load_w(0, wq_d)

            def load_consts():
                load_w(1, wk_d)
                load_w(2, wv_d)
                nc.sync.dma_start(tril_t[:], tril_d.ap()[:])
                nc.sync.dma_start(maskt[:], maskf_d.ap()[:])

            # Per-batch dual-head QKV storage ([128, L]; head h at
            # partitions 64h..64h+64).
            QT = [qkvp.tile([128, L], F32R, tag=f"qt{b}", name=f"qt{b}")
                  for b in range(B)]
            KT = [qkvp.tile([128, L], F32R, tag=f"kt{b}", name=f"kt{b}")
                  for b in range(B)]
            # V^T-augmented tiles, bf16: per (ki, h) a [128, 65] block at
            # col (2ki+h)*65; col 64 = mask value (denominator column).
            VA = [qkvp.tile([128, 2 * NK * 65], BF16, tag=f"va{b}",
                            name=f"va{b}") for b in range(B)]

            def va_mask_cols(b):
                va_k = VA[b][:].rearrange("p (k c) -> p k c", k=NK)
                for h in range(2):
                    nc.vector.tensor_copy(
                        va_k[:, :, 65 * h + 64:65 * h + 65],
                        maskt[:, b * NK:(b + 1) * NK].rearrange(
                            "p k -> p k ()"))

            def load_group(b, g, pieces=2):
                """One 512-token group's x tiles as a few fused DMAs — one
                HWDGE descriptor-gen per piece instead of 8 (625ns cadence
                each).  More pieces -> matmuls start earlier (ramp)."""
                g0 = b * L + g * 512
                xt = xp.tile([128, ND * 512], BF16, tag="xt", name="xt",
                             bufs=8)
                dper = ND // pieces
                w_ = dper * 512
                for h in range(pieces):
                    nc.sync.dma_start(
                        xt[:, h * w_:(h + 1) * w_].rearrange(
                            "p (d t) -> p d t", d=dper),
                        xT_d.ap()[dper * 128 * h:dper * 128 * (h + 1),
                                  g0:g0 + 512].rearrange(
                            "(d p) t -> p d t", p=128))
                return [xt[:, d * 512:(d + 1) * 512] for d in range(ND)]

            def queue_vdir(b, g):
                """V^T computed directly: out[tok, feat] accumulates
                stationary x-chunks [d,128tok] against moving W_V chunks
                [d,128feat] (bf16 = 1 cyc at 128 wide).  No VT staging, no
                transposes; the mask-scaled copy into VA reads PSUM from
                GPSIMD."""
                shared_pv = {}

                def mk(half):
                    def f():
                        xts = gshared[(b, g)]["x"]
                        if half == 0:
                            shared_pv["t"] = psp.tile(
                                [128, 512], F32, tag="proj", name="pvt")
                        pvt = shared_pv["t"]
                        for tl in range(2 * half, 2 * half + 2):
                            for d in range(ND):
                                nc.tensor.matmul(
                                    pvt[:, tl * 128:(tl + 1) * 128],
                                    xts[d][:, tl * 128:(tl + 1) * 128],
                                    w_tiles[2][:, d * E:(d + 1) * E],
                                    start=(tl == 0 and d == 0),
                                    stop=(d == ND - 1))
                        for tl in range(2 * half, 2 * half + 2):
                            for h in range(2):
                                ki = 4 * g + tl
                                nc.vector.tensor_scalar_mul(
                                    VA[b][:, (2 * ki + h) * 65:
                                           (2 * ki + h) * 65 + 64],
                                    pvt[:, tl * 128 + 64 * h:
                                        tl * 128 + 64 * h + 64],
                                    maskt[:, b * NK + ki:b * NK + ki + 1])
                    return f

                queue(f"b{b}g{g}vda", mk(0))
                queue(f"b{b}g{g}va", mk(1))

            # ---------------- background work pump ----------------------
            from collections import deque
            pending = deque()
            emitted = set()

            def pump(n=1):
                for _ in range(n):
                    if pending:
                        label, unit = pending.popleft()
                        unit()
                        emitted.add(label)

            def ensure(label):
                while pending and label not in emitted:
                    pump()

            def queue(label, fn):
                pending.append((label, fn))

            def inline_load(b, g):
                gshared[(b, g)] = {"x": load_group(b, g)}

            def queue_phase(b, g, p):
                def mk_half(half):
                    def f():
                        shared = gshared[(b, g)]
                        if half == 0:
                            shared[p] = psp.tile([128, 512], F32,
                                                 tag="proj", name="pj")
                        pj = shared[p]
                        for d in range(4 * half, 4 * half + 4):
                            nc.tensor.matmul(
                                pj[:], w_tiles[p][:, d * E:(d + 1) * E],
                                shared["x"][d][:],
                                start=(d == 0), stop=(d == ND - 1))
                        if half == 1:
                            dst = (QT, KT)[p][b]
                            with tc.high_priority(offset=200):
                                nc.vector.tensor_copy(
                                    dst[:, g * 512:(g + 1) * 512], pj[:])
                    return f

                for half in range(2):
                    queue(f"b{b}g{g}p{p}{'ab'[half]}", mk_half(half))

            gshared = {}

            # ---------------- attention unit ----------------------------
            def attention_unit(b, h, reverse, deps, post_deps=None,
                               hook0=None, hook_pre=None, lag=True):
                strips = _unit_schedule(reverse)
                pp = ppp.tile([128, UCOLS], BF16, tag="pp", name="pp")
                po = psp.tile([128, 1536], F32, tag="po", name="po")
                po_touched = [False, False, False]

                def po_slice(qt):
                    bank, j = divmod(qt, 7)
                    c0 = bank * 512 + j * 65
                    return po[:, c0:c0 + 65], bank

                st_tiles = [None] * NSTRIP

                def emit_S(s):
                    # Rank S matmuls ahead of recently-queued pump work in
                    # the tile list-scheduler: tanh(s) waits only on these.
                    # No boost in the first unit's ramp: there the emission
                    # order (g3 proj, g2 proj, S) is already the schedule.
                    boost = 0 if (reverse and s < 3) else 200
                    ctx_p = tc.high_priority(offset=boost)
                    ctx_p.__enter__()
                    st = psp.tile([128, 1024], F32, tag="strip", name="st",
                                  bufs=2)
                    st_tiles[s] = st
                    touched = [False, False]
                    for (c0, c1, ki, qs) in strips[s]["pieces"]:
                        o0 = c0 % 1024
                        bank = o0 // 512
                        nc.tensor.matmul(
                            st[:, o0:o0 + (c1 - c0)],
                            KT[b][h * 64:h * 64 + 64,
                                  ki * 128:(ki + 1) * 128],
                            QT[b][h * 64:h * 64 + 64, qs:qs + (c1 - c0)],
                            start=not touched[bank], stop=True)
                        touched[bank] = True
                    ctx_p.__exit__(None, None, None)

                def emit_tanh(s):
                    st = st_tiles[s]
                    nc.scalar.activation(st[:], st[:], AF.Tanh, scale=0.125)

                def emit_exp(s):
                    st = st_tiles[s]
                    nc.scalar.activation(pp[:, s * 1024:(s + 1) * 1024],
                                         st[:], AF.Exp, bias=n30_t[:],
                                         scale=TAU)
                    st_tiles[s] = None

                def emit_post(s):
                    for (c, ki) in strips[s]["diag"]:
                        nc.vector.tensor_mul(pp[:, c:c + 128],
                                             pp[:, c:c + 128], tril_t[:])
                    for (c, ki, qt) in strips[s]["av"]:
                        dst, bank = po_slice(qt)
                        stop = (ki == qt) if not reverse else (ki == 0)
                        nc.tensor.matmul(
                            dst, pp[:, c:c + 128],
                            VA[b][:, (2 * ki + h) * 65:(2 * ki + h) * 65 + 65],
                            start=not po_touched[bank], stop=stop)
                        po_touched[bank] = True
                    for qt in strips[s]["done"]:
                        src, _ = po_slice(qt)
                        rec = epip.tile([128, 1], F32, tag="rec", name="rec")
                        nc.vector.reciprocal(rec[:], src[:, 64:65])
                        nc.vector.tensor_scalar_mul(
                            ostages[b][qt][:, h * 64:(h + 1) * 64],
                            src[:, 0:64], rec[:])
                        if h == 1:
                            nc.sync.dma_start(
                                out_d.ap()[b, qt * 128:(qt + 1) * 128, :],
                                ostages[b][qt][:])

                # Strips are processed in PAIRS with the ACT stream ordered
                # tanh(s), tanh(s+1), exp(s), exp(s+1): the RAW tanh->exp
                # semaphore wait (~177ns pipeline tail) then lands inside the
                # neighboring activation instead of idling the ACT engine.
                # The next pair's S matmuls are emitted first (each engine
                # runs its program in order; tanh waits only on S).
                # The post phase (tril + AV + drains) of each strip group is
                # emitted one group LATE: AV blocks never gate the ACT
                # stream, and the lag keeps the V-phase/va work (which AV
                # needs) off the S-matmul critical path during the ramp.
                groups = [[0]] + [[s, s + 1] for s in range(1, NSTRIP, 2)]
                post_deps = post_deps or {}

                def do_post(s):
                    for lbl in post_deps.get(s, []):
                        ensure(lbl)
                    emit_post(s)

                for i, grp in enumerate(groups):
                    nxt = groups[i + 1] if i + 1 < len(groups) else []
                    if i == 0:
                        if hook0 is not None:
                            hook0()
                        for s in grp:
                            for lbl in deps.get(s, []):
                                ensure(lbl)
                            emit_S(s)
                    if lag and i > 0:
                        for s in groups[i - 1]:
                            do_post(s)
                    for s in nxt:
                        for lbl in deps.get(s, []):
                            ensure(lbl)
                        emit_S(s)
                    pump()
                    for s in grp:
                        emit_tanh(s)
                    if lag:
                        for s in grp:
                            emit_exp(s)
                    else:
                        # last unit: post each strip right after its exp so
                        # the final drain/store chain starts as early as
                        # possible (shorter tail)
                        for s in grp:
                            emit_exp(s)
                            do_post(s)
                    pump()
                if lag:
                    for s in groups[-1]:
                        do_post(s)

            ostages = [[ostagep.tile([128, 128], F32, tag=f"os{b}_{j}",
                                     name=f"os{b}_{j}")
                        for j in range(NK)] for b in range(B)]

            # ---------------- prologue: b0 groups 3, 2 -------------------
            # Q|K packed into one 2-bank strip tile; V in the proj bank;
            # then va transposes reuse the proj bank.
            # PE warmup: the cost model's p-state needs ~3us of continuous
            # matmul activity to reach full clock.  Burn idle PE time at
            # t=0 on throwaway matmuls so the first projections run warm.
            warm_sb = constp.tile([128, 256], F32R, tag="warm")
            nc.gpsimd.memset(warm_sb[:].bitcast(F32), 0.0)
            warm_ps = psp.tile([128, 512], F32, tag="proj", name="warm")
            for i in range(10):
                nc.tensor.matmul(warm_ps[:, 0:256], warm_sb[:, 0:128],
                                 warm_sb[:], start=True, stop=True)

            # Prologue: group 3 of batch 0 inline — Q|K packed into one
            # 2-bank strip tile, pipelined with the fused x DMAs; V and
            # the va transposes are deferred to the pump queue so the
            # first attention strips start as early as possible.
            def prologue_qk(b, g, act_drain=False):
                xts = load_group(b, g, pieces=4)
                gshared[(b, g)] = {"x": xts}
                st = psp.tile([128, 1024], F32, tag="strip", name="pqk",
                              bufs=2)
                for d in range(ND):
                    nc.tensor.matmul(st[:, 0:512],
                                     w_tiles[0][:, d * E:(d + 1) * E],
                                     xts[d][:], start=(d == 0),
                                     stop=(d == ND - 1))
                    nc.tensor.matmul(st[:, 512:1024],
                                     w_tiles[1][:, d * E:(d + 1) * E],
                                     xts[d][:], start=(d == 0),
                                     stop=(d == ND - 1))
                nc.vector.tensor_copy(QT[b][:, g * 512:(g + 1) * 512],
                                      st[:, 0:512])
                if act_drain:
                    # ACT is idle before the first strip: drain K there so
                    # it runs in parallel with Q's DVE drain.
                    nc.scalar.activation(KT[b][:, g * 512:(g + 1) * 512],
                                         st[:, 512:1024], AF.Identity)
                else:
                    nc.vector.tensor_copy(KT[b][:, g * 512:(g + 1) * 512],
                                          st[:, 512:1024])

            load_w(1, wk_d)
            prologue_qk(0, 3)
            nc.sync.dma_start(tril_t[:], tril_d.ap()[:])
            nc.sync.dma_start(maskt[:], maskf_d.ap()[:])
            va_mask_cols(0)
            va_mask_cols(1)

            # group 2 (packed QK) is emitted from inside unit 0, right
            # after strip 0's S matmuls, so S(0) stays ahead of it on the
            # in-order PE stream.
            def hook_g2():
                prologue_qk(0, 2)
                load_w(2, wv_d)
                inline_load(0, 1)
                inline_load(0, 0)
                for g in range(NG):
                    inline_load(1, g)

            # All remaining x loads go on the DMA queue back-to-back, up
            # front (the DMA device is the ramp bottleneck; consumers wait
            # on semaphores).  The pump queue holds only compute: g1's Q/K
            # before the g3/g2 V-phase work (S matmuls gate ACT; AV work
            # is post-lagged), b1 Q phases before b1 K/V/va.
            queue_phase(0, 1, 0)
            queue_phase(0, 1, 1)
            queue_phase(0, 0, 0)
            queue_phase(0, 0, 1)
            queue_vdir(0, 3)
            queue_vdir(0, 2)
            queue_vdir(0, 1)
            queue_vdir(0, 0)
            for g in range(NG):
                queue_phase(1, g, 0)
            for g in range(NG):
                queue_phase(1, g, 1)
                queue_vdir(1, g)

            rev_deps = {4: ["b0g1p1b"], 9: ["b0g0p1b"]}
            rev_post = {0: ["b0g3va"], 1: ["b0g2va"],
                        4: ["b0g1va"], 9: ["b0g0va"]}
            u2_deps = {0: ["b1g0p1b"], 7: ["b1g1p1b"],
                       12: ["b1g2p1b"], 15: ["b1g3p1b"]}
            u2_post = {0: ["b1g0va"], 7: ["b1g1va"],
                       12: ["b1g2va"], 15: ["b1g3va"]}

            attention_unit(0, 0, reverse=True, deps=rev_deps,
                           post_deps=rev_post, hook0=hook_g2)
            attention_unit(0, 1, reverse=False, deps={})
            attention_unit(1, 0, reverse=False, deps=u2_deps,
                           post_deps=u2_post)
            attention_unit(1, 1, reverse=False, deps={}, lag=False)
            while pending:
                pump()

    nc.compile()
    return nc


def _get_program():
    if "nc" not in _CACHE:
        _CACHE["nc"] = _build_program()
    return _CACHE["nc"]


def _wprep(W, sl):
    """Per-core weight slice in the SBUF tile layout [128, 8*128]
    (partition-major: wt[p, d*128+e] = W.T[128*d+p, e]) so the DMA is a
    straight wide-row copy (2KB runs, no small-element penalty)."""
    import ml_dtypes
    wt = np.asarray(W, np.float32)[sl, :].T.reshape(8, 128, 128)
    wt = wt.transpose(1, 0, 2).reshape(128, 1024)
    return np.ascontiguousarray(wt).astype(ml_dtypes.bfloat16)


def _prep_inputs(input, attention_mask, W_Q, W_K, W_V):
    import ml_dtypes

    x = np.asarray(input, dtype=np.float32).reshape(T, D)
    xT = np.ascontiguousarray(x.T).astype(ml_dtypes.bfloat16)   # [D, T]
    mask = np.asarray(attention_mask).astype(np.float32)    # [B, L]
    # maskf[p, b*16+ki] = mask[b, 128*ki + p]
    maskf = np.ascontiguousarray(
        mask.reshape(B, NK, 128).transpose(2, 0, 1).reshape(128, B * NK))
    tril = np.triu(np.ones((128, 128), dtype=np.float32))   # keep[k, q] = q >= k
    tril = tril.astype(ml_dtypes.bfloat16)
    common = {"xT": xT, "tril": tril, "maskf": maskf}
    in_maps = []
    for c in range(N_CORES):
        sl = slice(c * E, (c + 1) * E)
        in_maps.append({
            **common,
            "wq": _wprep(W_Q, sl),
            "wk": _wprep(W_K, sl),
            "wv": _wprep(W_V, sl),
        })
    return in_maps


def kernel(input, attention_mask, W_Q, W_K, W_V):
    from concourse.bass_utils import run_bass_kernel_spmd

    nc = _get_program()
    in_maps = _prep_inputs(input, attention_mask, W_Q, W_K, W_V)
    res = run_bass_kernel_spmd(nc, in_maps, list(range(N_CORES)))
    return np.concatenate([res.results[c]["out"] for c in range(N_CORES)],
                          axis=2)
